# revision 25
# baseline (speedup 1.0000x reference)
"""Causal GQA attention (B=2, S=2048, HID=2048, H=16, KVH=4, D=128) on 8 TRN2 cores.

Sharding: core c -> batch c//4, kv-group c%4 (4 q-heads + 1 kv-head).
o_proj is row-split by head group; host sums the 4 partials per batch.

Device kernel (per core, bf16 matmuls / f32 accumulation):
  xT (host-pretransposed [HID, S]) -> v+k projections fused in one ko-outer
  loop (both consume each xT chunk as it lands) -> RoPE (pair-permuted on host
  into wq/wk columns, applied via half-swap + cos/sin muls) -> q projections ->
  attention in 512-wide q-blocks (every PSUM tile = 1 bank): scores^T = K.Q^T,
  exp on ScalarE, causal tri-mask on the diagonal 128-block, AV^T with
  V-natural stationary, softmax denominator via a 128-row all-ones matmul
  (output pre-broadcast across partitions -> no gpsimd), 2-iteration software
  pipeline, normalize via reciprocal_approx_fast ->
  o_proj (out[t,o] += avN_h.T @ wo_h).
Diagonal tiles only compute the valid q-range [128r:512].
"""

import numpy as np
import ml_dtypes

BF16 = ml_dtypes.bfloat16

B, S, HID = 2, 2048, 2048
H, KVH, D = 16, 4, 128
P = 128
KO = HID // P          # 16 contraction tiles
HQ = H // KVH          # 4 q heads per core
NTB = S // 512         # 4 token tiles of 512 (projection)
NQB = S // 512         # 4 query blocks of 512 (attention)
QW = 512
NKB = S // P           # 16 key blocks of 128
N_CORES = 8
WARMUP = 30

_CACHE = {}


def _build_nc():
    import concourse.tile as tile
    from concourse import bacc, mybir
    from concourse.masks import make_identity
    from contextlib import ExitStack

    bf = mybir.dt.bfloat16
    f32 = mybir.dt.float32
    AF = mybir.ActivationFunctionType
    QS = 2 * QW  # 1024-wide projection supertile

    nc = bacc.Bacc("TRN2", target_bir_lowering=False, debug=False,
                   num_devices=N_CORES)

    xT_d = nc.dram_tensor("xT", [HID, S], bf, kind="ExternalInput").ap()
    wq_d = nc.dram_tensor("wq", [P, KO * HQ * D], bf, kind="ExternalInput").ap()
    wk_d = nc.dram_tensor("wk", [P, KO * D], bf, kind="ExternalInput").ap()
    wv_d = nc.dram_tensor("wv", [P, KO * D], bf, kind="ExternalInput").ap()
    wo_d = nc.dram_tensor("wo", [P, HQ * HID], bf, kind="ExternalInput").ap()
    cs_d = nc.dram_tensor("cs2", [P, S], bf, kind="ExternalInput").ap()
    ss_d = nc.dram_tensor("ss2", [P, S], bf, kind="ExternalInput").ap()
    mk_d = nc.dram_tensor("mask", [P, P], bf, kind="ExternalInput").ap()
    out_d = nc.dram_tensor("out", [S, HID], bf, kind="ExternalOutput").ap()
    out_r = out_d.rearrange("(tb p) o -> p tb o", p=P)

    with tile.TileContext(nc) as tc:
        with ExitStack() as octx:
            const = octx.enter_context(tc.tile_pool(name="const", bufs=1))
            rope_p = octx.enter_context(tc.tile_pool(name="rope", bufs=3))
            at_p = octx.enter_context(tc.tile_pool(name="at", bufs=8))
            nrm = octx.enter_context(tc.tile_pool(name="nrm", bufs=2))
            xt_ctx = ExitStack()
            xt_pool = xt_ctx.enter_context(tc.tile_pool(name="xt", bufs=1))

            # warmup constants first: DVE memsets only, so the PE warmup can
            # start the instant engines come up (no DMA/gpsimd dependency)
            ones = const.tile([P, 1], bf, tag="ones", name="ones")
            nc.vector.memset(ones[:], 1.0)
            ones128 = const.tile([P, P], bf, tag="ones128", name="ones128")
            nc.vector.memset(ones128[:], 1.0)

            # ---- persistent loads (order matters: v/k weights before xT) ----
            # weights DMA'd flat (contiguous per-partition descriptors — a
            # DRAM-side rearrange would shatter them into 256B pieces); the
            # SBUF APs are rearranged for matmul use instead
            wv_flat = const.tile([P, KO * D], bf, tag="wv", name="wv")
            nc.sync.dma_start(wv_flat[:], wv_d[:])
            wv_sb = wv_flat.rearrange("p (ko n) -> p ko n", ko=KO)
            xTk = [xt_pool.tile([P, S], bf, tag=f"xT{ko}", name=f"xT{ko}")
                   for ko in range(KO)]
            # first chunk lands quarter-by-quarter so the fused v+k loop can
            # start on quarter 0 ~3us sooner
            nc.sync.dma_start(xTk[0][:, 0:QW], xT_d[0:P, 0:QW])
            nc.sync.dma_start(xTk[0][:, QW:2 * QW], xT_d[0:P, QW:2 * QW])
            wk_flat = const.tile([P, KO * D], bf, tag="wk", name="wk")
            nc.sync.dma_start(wk_flat[:], wk_d[:])
            wk_sb = wk_flat.rearrange("p (ko n) -> p ko n", ko=KO)
            nc.sync.dma_start(xTk[0][:, 2 * QW:S], xT_d[0:P, 2 * QW:S])
            for ko in range(1, KO):
                nc.sync.dma_start(xTk[ko][:], xT_d[ko * P:(ko + 1) * P, :])
            cs_sb = const.tile([P, S], bf, tag="cs", name="cs")
            nc.sync.dma_start(cs_sb[:], cs_d[:])
            ss_sb = const.tile([P, S], bf, tag="ss", name="ss")
            nc.sync.dma_start(ss_sb[:], ss_d[:])
            wq_flat = const.tile([P, KO * HQ * D], bf, tag="wq", name="wq")
            nc.sync.dma_start(wq_flat[:], wq_d[:])
            wq_sb = wq_flat.rearrange("p (ko n) -> p ko n", ko=KO)
            mk_sb = const.tile([P, P], bf, tag="mk", name="mk")
            nc.sync.dma_start(mk_sb[:], mk_d[:])
            wo_flat = const.tile([P, HQ * HID], bf, tag="wo", name="wo")
            nc.sync.dma_start(wo_flat[:], wo_d[:])
            wo_sb = wo_flat.rearrange("p (h o) -> p h o", h=HQ)
            ident = const.tile([P, P], bf, tag="ident", name="ident")
            make_identity(nc, ident[:])

            qR = [const.tile([P, S], bf, tag=f"qR{h}", name=f"qR{h}")
                  for h in range(HQ)]
            kR = const.tile([P, S], bf, tag="kR", name="kR")
            vT_sb = const.tile([P, S], bf, tag="vT", name="vT")
            vN = const.tile([P, NKB, D], bf, tag="vN", name="vN")
            avN = [const.tile([P, S], bf, tag=f"avN{h}", name=f"avN{h}")
                   for h in range(HQ)]

            def rope_tile(ps, out_sl, tb):
                tsl = slice(tb * QW, (tb + 1) * QW)
                raw = rope_p.tile([P, QW], bf, tag="rp_raw", name="rp_raw")
                nc.vector.tensor_copy(raw[:], ps[:])
                sw = rope_p.tile([P, QW], bf, tag="rp_sw", name="rp_sw")
                nc.sync.dma_start(sw[0:64, :], raw[64:128, :])
                nc.sync.dma_start(sw[64:128, :], raw[0:64, :])
                t1 = rope_p.tile([P, QW], bf, tag="rp_t1", name="rp_t1")
                nc.vector.tensor_mul(t1[:], raw[:], cs_sb[:, tsl])
                t2 = rope_p.tile([P, QW], bf, tag="rp_t2", name="rp_t2")
                nc.vector.tensor_mul(t2[:], sw[:], ss_sb[:, tsl])
                nc.vector.tensor_add(out_sl, t1[:], t2[:])

            # ================= projection phase =================
            with ExitStack() as pctx:
                # PSUM: tag "s" = [P,1024]f32 (2 banks) x 4 bufs -> all 8 banks
                ps_p = pctx.enter_context(
                    tc.tile_pool(name="ps_p", bufs=4, space="PSUM"))

                # HAM warmup: keep PE busy so the clock gate opens before
                # the DMA-paced projections start
                junk = ps_p.tile([P, QS], f32, tag="s", name="junk")
                for _ in range(WARMUP):
                    nc.tensor.matmul(junk[0:1, 0:P], lhsT=ones[:],
                                     rhs=ones128[:], start=True, stop=True)

                # ---- fused v+k projection (ko-outer): both consume each
                # xT chunk as it lands, halving DMA-paced PE idle ----
                va = ps_p.tile([P, QS], f32, tag="s", name="va")
                vb = ps_p.tile([P, QS], f32, tag="s", name="vb")
                ka = ps_p.tile([P, QS], f32, tag="s", name="ka")
                kb4 = ps_p.tile([P, QS], f32, tag="s", name="kb4")
                qv = [va[:, 0:QW], va[:, QW:QS], vb[:, 0:QW], vb[:, QW:QS]]
                qk = [ka[:, 0:QW], ka[:, QW:QS], kb4[:, 0:QW], kb4[:, QW:QS]]
                for ko in range(KO):
                    for i in range(4):
                        nc.tensor.matmul(
                            qv[i][:D, :], lhsT=wv_sb[:, ko, :],
                            rhs=xTk[ko][:, i * QW:(i + 1) * QW],
                            start=(ko == 0), stop=(ko == KO - 1))
                    for i in range(4):
                        nc.tensor.matmul(
                            qk[i][:D, :], lhsT=wk_sb[:, ko, :],
                            rhs=xTk[ko][:, i * QW:(i + 1) * QW],
                            start=(ko == 0), stop=(ko == KO - 1))
                HW2 = QW // 2
                for tb in range(NTB):  # half-copies split across engines so
                    # the psum ring slot q-proj needs first frees in ~0.35us
                    t0 = tb * QW
                    nc.scalar.copy(vT_sb[:, t0:t0 + HW2], qv[tb][:, 0:HW2])
                    nc.vector.tensor_copy(vT_sb[:, t0 + HW2:t0 + QW],
                                          qv[tb][:, HW2:QW])
                for tb in range(NTB):
                    rope_tile(qk[tb], kR[:, tb * QW:(tb + 1) * QW], tb)

                # ---- q projections ----
                for h in range(HQ):
                    for pair in range(2):
                        t = ps_p.tile([P, QS], f32, tag="s", name="qps")
                        halves = [t[:, 0:QW], t[:, QW:QS]]
                        for ko in range(KO):
                            for i in range(2):
                                tb = 2 * pair + i
                                nc.tensor.matmul(
                                    halves[i][:D, :],
                                    lhsT=wq_sb[:, ko, h * D:(h + 1) * D],
                                    rhs=xTk[ko][:, tb * QW:(tb + 1) * QW],
                                    start=(ko == 0), stop=(ko == KO - 1))
                        for i in range(2):
                            tb = 2 * pair + i
                            rope_tile(halves[i],
                                      qR[h][:, tb * QW:(tb + 1) * QW], tb)

                # ---- v transpose to natural layout (after q proj: the vT
                # staging copies overlap q-proj matmuls instead of stalling
                # the PE) ----
                for kb in range(NKB):
                    pst = ps_p.tile([P, P], bf, tag="s", name="pst")
                    nc.tensor.transpose(
                        pst[:], vT_sb[:, kb * P:(kb + 1) * P], ident[:])
                    nc.scalar.copy(vN[:, kb, :], pst[:])

            xt_ctx.close()  # xT tiles dead; frees SBUF

            # ================= attention phase =================
            # 512-wide q-blocks: every PSUM tile is exactly 1 bank.
            # s x4 + av x2 + dn x2 = 8 banks.
            with ExitStack() as ctx:
                ps_s_p = ctx.enter_context(
                    tc.tile_pool(name="ps_s", bufs=4, space="PSUM"))
                ps_av_p = ctx.enter_context(
                    tc.tile_pool(name="ps_av", bufs=2, space="PSUM"))
                ps_dn_p = ctx.enter_context(
                    tc.tile_pool(name="ps_dn", bufs=2, space="PSUM"))

                seq = [(h, jq, kb)
                       for h in range(HQ)
                       for jq in range(NQB)
                       for kb in range(4 * jq + 4)]
                st = {}

                def scores_i(h, jq, kb):
                    q0 = jq * QW
                    if kb == 0:
                        st[(h, jq)] = {"ats": {}}
                    s = st[(h, jq)]
                    r = kb - 4 * jq
                    lo = 128 * r if r >= 0 else 0
                    ps_s = ps_s_p.tile([P, QW], f32, tag="s", name="s")
                    nc.tensor.matmul(
                        ps_s[:, lo:QW],
                        lhsT=kR[:, kb * P:(kb + 1) * P],
                        rhs=qR[h][:, q0 + lo:q0 + QW],
                        start=True, stop=True)
                    at = at_p.tile([P, QW], bf, tag="at", name="at")
                    nc.scalar.activation(at[:, lo:QW], ps_s[:, lo:QW], AF.Exp)
                    if r >= 0:
                        nc.vector.tensor_mul(
                            at[:, lo:lo + P], at[:, lo:lo + P], mk_sb[:])
                    s["ats"][kb] = at

                def accum_i(h, jq, kb):
                    nkb = 4 * jq + 4
                    q0 = jq * QW
                    s = st[(h, jq)]
                    if kb == 0:
                        s["av"] = ps_av_p.tile([P, QW], f32, tag="av",
                                               name="av")
                        s["dn"] = ps_dn_p.tile([P, QW], f32, tag="dn",
                                               name="dn")
                    r = kb - 4 * jq
                    lo = 128 * r if r >= 0 else 0
                    at = s["ats"].pop(kb)
                    nc.tensor.matmul(
                        s["av"][:, lo:QW], lhsT=vN[:, kb, :],
                        rhs=at[:, lo:QW],
                        start=(kb == 0), stop=(kb == nkb - 1))
                    # denominator: tree-sum each at-quad on DVE (bf16 2x
                    # rate) into the quad's first tile, so the PE streams a
                    # quarter of the dn columns. AV(kb) above was issued
                    # first, so the in-place adds can't corrupt its rhs read.
                    # at(kb+1) always exists: scores run 3 iterations ahead.
                    qphase = kb % 4

                    def _lo(x):
                        rr = x - 4 * jq
                        return 128 * rr if rr > 0 else 0
                    if qphase == 0:
                        # in-place adds target ONLY the quad's first tile;
                        # the later tiles are read-only (read-read with
                        # their own AV matmuls is safe). One add per
                        # iteration keeps a full-iteration slack ahead of
                        # the dn matmul.
                        at1 = s["ats"][kb + 1]
                        nc.vector.tensor_add(
                            at[:, _lo(kb + 1):QW], at[:, _lo(kb + 1):QW],
                            at1[:, _lo(kb + 1):QW])
                        s["quad"] = (at, lo)
                    elif qphase == 1:
                        at0, lo0 = s["quad"]
                        at2 = s["ats"][kb + 1]
                        nc.vector.tensor_add(
                            at0[:, _lo(kb + 1):QW], at0[:, _lo(kb + 1):QW],
                            at2[:, _lo(kb + 1):QW])
                    elif qphase == 2:
                        at0, lo0 = s["quad"]
                        at3 = s["ats"][kb + 1]
                        nc.vector.tensor_add(
                            at0[:, _lo(kb + 1):QW], at0[:, _lo(kb + 1):QW],
                            at3[:, _lo(kb + 1):QW])
                    elif qphase == 3:
                        at0, lo0 = s.pop("quad")
                        # 128-row all-ones lhsT: output rows pre-broadcast
                        # across partitions (no gpsimd broadcast needed)
                        nc.tensor.matmul(
                            s["dn"][:, lo0:QW], lhsT=ones128[:],
                            rhs=at0[:, lo0:QW],
                            start=(kb == 3), stop=(kb == nkb - 1))
                    if kb == nkb - 1:
                        rcp = nrm.tile([P, QW], f32, tag="rcp", name="rcp")
                        nc.vector.reciprocal_approx_fast(rcp[:], s["dn"][:])
                        nc.vector.tensor_mul(
                            avN[h][:, q0:q0 + QW], s["av"][:], rcp[:])
                        del st[(h, jq)]

                # 3-iteration software pipeline: scores run 3 tiles ahead of
                # AV so ScalarE exp latency never stalls the PE.
                scores_i(*seq[0])
                scores_i(*seq[1])
                scores_i(*seq[2])
                for i in range(3, len(seq)):
                    scores_i(*seq[i])
                    accum_i(*seq[i - 3])
                accum_i(*seq[-3])
                accum_i(*seq[-2])
                accum_i(*seq[-1])

            # ================= o_proj =================
            # quarter-granular psum ([P,512] = 1 bank x 8 bufs): each quarter
            # drains with one copy + one DMA right after its 4 hh-matmuls, so
            # the final drain is ~1 copy + 1 DMA instead of a 4-copy chain
            with ExitStack() as ctx:
                op_p = ctx.enter_context(
                    tc.tile_pool(name="op", bufs=8, space="PSUM"))
                ost_p = ctx.enter_context(tc.tile_pool(name="ost", bufs=6))
                qi = 0
                for tb in range(NKB):  # 16 token tiles of 128
                    for ob in range(4):
                        pso = op_p.tile([P, QW], f32, tag="o", name="o")
                        for hh in range(HQ):
                            nc.tensor.matmul(
                                pso[:],
                                lhsT=avN[hh][:, tb * P:(tb + 1) * P],
                                rhs=wo_sb[:, hh, ob * QW:(ob + 1) * QW],
                                start=(hh == 0), stop=(hh == HQ - 1))
                        ot = ost_p.tile([P, QW], bf, tag="ot", name="ot")
                        if qi % 2 == 0:
                            nc.scalar.copy(ot[:], pso[:])
                        else:
                            nc.vector.tensor_copy(ot[:], pso[:])
                        qi += 1
                        nc.sync.dma_start(
                            out_r[:, tb, ob * QW:(ob + 1) * QW], ot[:])

    nc.compile()
    return nc


def _prep_inputs(x, freqs_cis, wq, wk, wv, wo):
    x = np.asarray(x, dtype=np.float32)
    freqs = np.asarray(freqs_cis, dtype=np.float32)
    wq = np.asarray(wq, dtype=np.float32)
    wk = np.asarray(wk, dtype=np.float32)
    wv = np.asarray(wv, dtype=np.float32)
    wo = np.asarray(wo, dtype=np.float32)

    perm = np.concatenate([np.arange(0, D, 2), np.arange(1, D, 2)])
    cos = freqs[..., 0].T.astype(np.float32)            # [64, S]
    sin = freqs[..., 1].T.astype(np.float32)
    cs2 = np.ascontiguousarray(np.concatenate([cos, cos], 0)).astype(BF16)
    ss2 = np.ascontiguousarray(np.concatenate([-sin, sin], 0)).astype(BF16)

    wq_p = (wq.reshape(HID, H, D)[:, :, perm] * D**-0.5).astype(BF16)
    wk_p = wk.reshape(HID, KVH, D)[:, :, perm].astype(BF16)
    wv_r = wv.reshape(HID, KVH, D).astype(BF16)
    wo_r = wo.reshape(H, D, HID)

    kk = np.arange(P)[:, None]
    qq = np.arange(P)[None, :]
    tri = (kk <= qq).astype(BF16)                        # [128, 128]

    xT = np.ascontiguousarray(x.transpose(0, 2, 1)).astype(BF16)  # [B, HID, S]

    def swz(w):  # [HID, N] -> [P, KO*N] so each partition's DMA is contiguous
        n = w.shape[1]
        return np.ascontiguousarray(
            w.reshape(KO, P, n).transpose(1, 0, 2).reshape(P, KO * n))

    in_maps = []
    for c in range(N_CORES):
        b, g = c // 4, c % 4
        wo_g = wo_r[4 * g:4 * g + HQ].astype(BF16)      # [HQ, P, HID]
        in_maps.append({
            "xT": xT[b],
            "wq": swz(wq_p[:, 4 * g:4 * g + HQ, :].reshape(HID, HQ * D)),
            "wk": swz(wk_p[:, g, :]),
            "wv": swz(wv_r[:, g, :]),
            "wo": np.ascontiguousarray(
                wo_g.transpose(1, 0, 2).reshape(P, HQ * HID)),
            "cs2": cs2,
            "ss2": ss2,
            "mask": tri,
        })
    return in_maps


def _ensure_ntff_hook():
    """Optional: register the NTFF profiling hook if the image's antenv lacks
    it, so BASS_TRACE=1 produces a profile instead of crashing. No-op on
    failure or when the hook already exists."""
    import sys as _sys
    import types as _types
    try:
        from antenv.axon_hooks import get_axon_ntff_profile_hook  # noqa: F401
        return
    except ImportError:
        pass
    try:
        from trn_agent_boot.trn_boot import _ntff_profile_via_ctypes
        hook = _ntff_profile_via_ctypes("/opt/axon/libaxon_pjrt.so")
        mod = _types.ModuleType("antenv.axon_hooks")
        mod.get_axon_ntff_profile_hook = lambda: hook
        mod.set_axon_ntff_profile_hook = lambda h: None
        _sys.modules["antenv.axon_hooks"] = mod
    except Exception:
        pass


def kernel(x, freqs_cis, wq, wk, wv, wo):
    from concourse.bass_utils import run_bass_kernel_spmd
    _ensure_ntff_hook()

    nc = _CACHE.get("nc")
    if nc is None:
        nc = _build_nc()
        _CACHE["nc"] = nc

    in_maps = _prep_inputs(x, freqs_cis, wq, wk, wv, wo)
    res = run_bass_kernel_spmd(nc, in_maps, list(range(N_CORES)))
    _CACHE["last_result"] = res
    parts = [np.asarray(res.results[c]["out"]).astype(np.float32)
             for c in range(N_CORES)]
    out = np.stack([parts[0] + parts[1] + parts[2] + parts[3],
                    parts[4] + parts[5] + parts[6] + parts[7]])
    return out
